# revision 37
# baseline (speedup 1.0000x reference)
"""Trainium2 Bass kernel for Mllama-style GQA self-attention (B=1, S=2048,
H=32 q-heads, KVH=8 kv-heads, D=128), tensor-parallel over heads across 8
NeuronCores.

Sharding: core c owns kv-head c and q-heads 4c..4c+3 (Wq/Wk/Wv column shards),
computes its heads' attention output in transposed [feature, seq] layout, then
computes a partial output projection over its 512 features producing full
4096-wide rows in bf16; the host sums the 8 cores' partials in fp32.

Global software pipeline: attention for seq-chunk c (scores, exp, PV) is
emitted interleaved into phase-1 chunk c+1's projection matmul stream, so the
exp/mask latency chains hide under independent PE work and the PE stays
continuously busy (keeping its p-state at max). Scores are issued in 2-block
pairs into a 2-bank PSUM tile so one Act-engine exp instruction covers both
blocks. o_proj for chunks 0-2 interleaves with chunk 3's attention; softmax
normalization uses reciprocal_approx_fast and a GpSimd partition broadcast;
diagonal (causal-boundary) blocks compute only their valid column range.
"""
import math
from contextlib import ExitStack
import numpy as np
import ml_dtypes

import concourse.bass as bass
import concourse.bacc as bacc
import concourse.mybir as mybir
import concourse.tile as tile
from concourse.bass_utils import run_bass_kernel_spmd

BF16 = ml_dtypes.bfloat16
S, E, H, KVH, D = 2048, 4096, 32, 8, 128
N_CORES = 8
G = H // KVH                      # q heads per core
OSH = G * D                       # per-core q/attn feature count (512)
PC = 512                          # phase-1 seq chunk (projection rhs width)
AC = 512                          # attention sq chunk width
N_PC = S // PC                    # 4
N_AC = S // AC                    # 4
NE = E // 128                     # 32 contraction tiles
N_ST = S // 128                   # 16 seq tiles

_BUILD_CACHE = {}


def build_bass(causal: bool):
    key = causal
    if key in _BUILD_CACHE:
        return _BUILD_CACHE[key]
    dt = mybir.dt
    nc = bacc.Bacc("TRN2", target_bir_lowering=False, debug=False,
                   enable_asserts=False, num_devices=N_CORES)

    XT4 = nc.dram_tensor("xt", [N_PC, 128, NE, PC], dt.bfloat16, kind="ExternalInput").ap()
    WQT = nc.dram_tensor("wqt", [128, G, NE, D], dt.bfloat16, kind="ExternalInput").ap()
    WKT = nc.dram_tensor("wkt", [128, NE, D], dt.bfloat16, kind="ExternalInput").ap()
    WVT = nc.dram_tensor("wvt", [128, NE, D], dt.bfloat16, kind="ExternalInput").ap()
    WOT = nc.dram_tensor("wot", [128, G, E], dt.bfloat16, kind="ExternalInput").ap()
    ROPE = nc.dram_tensor("rope", [4, D, S], dt.bfloat16, kind="ExternalInput").ap()
    TRI = nc.dram_tensor("tri", [4, 128, AC], dt.bfloat16, kind="ExternalInput").ap()
    OUT = nc.dram_tensor("out", [S, E], dt.bfloat16, kind="ExternalOutput").ap()

    with tile.TileContext(nc) as tc:
        with (
            tc.tile_pool(name="wpool", bufs=1) as wpool,
            tc.tile_pool(name="qkv", bufs=1) as qkvpool,
            tc.tile_pool(name="consts", bufs=1) as cpool,
            tc.tile_pool(name="epool", bufs=13) as epool,
            tc.tile_pool(name="esump", bufs=3) as esumpool,
            tc.tile_pool(name="recipp", bufs=2) as recippool,
            tc.tile_pool(name="bcp", bufs=2) as bcpool,
            tc.tile_pool(name="attn", bufs=16) as attnpool,
            tc.tile_pool(name="ps_st", bufs=1, space="PSUM") as ps_st,
            tc.tile_pool(name="ps_ot", bufs=2, space="PSUM") as ps_ot,
            tc.tile_pool(name="ps_den", bufs=1, space="PSUM") as ps_den,
        ):
            # phase-1-only pools; closed after phase 1 so wo/e3/out pools
            # can reuse their SBUF space (and ps_po the PSUM bank pair)
            p1ctx = ExitStack()
            xspool = p1ctx.enter_context(tc.tile_pool(name="xs", bufs=2))
            cspool = p1ctx.enter_context(tc.tile_pool(name="cs", bufs=2))
            rtmppool = p1ctx.enter_context(tc.tile_pool(name="rtmp", bufs=1))
            ps_p1 = p1ctx.enter_context(
                tc.tile_pool(name="ps_p1", bufs=2, space="PSUM"))

            # --- resident weights. Priority: first chunk's activations and
            # Wk jump ahead of the big weight burst so the PE starts early.
            wq_sb = wpool.tile([128, G, NE, D], dt.bfloat16)
            wk_sb = wpool.tile([128, NE, D], dt.bfloat16)
            wv_sb = wpool.tile([128, NE, D], dt.bfloat16)

            xs0 = xspool.tile([128, NE, PC], dt.bfloat16, tag="xs")
            cs0 = cspool.tile([128, 4, PC], dt.bfloat16, tag="cs")
            # startup priority: k chain's tiles first, then Wq head-major
            # (chunk-0 chains run k, q0..q3, v: head 0's contiguous slice
            # gates first, later heads pipeline in, wv can arrive last)
            for q in range(4):
                nc.sync.dma_start(xs0[:, q:q + 1, :], XT4[0, :, q:q + 1, :])
                nc.sync.dma_start(wk_sb[:, q:q + 1, :], WKT[:, q:q + 1, :])
            nc.sync.dma_start(cs0[:, 2:4, :],
                              ROPE[2:4, :, 0:PC].rearrange("j p s -> p j s"))
            nc.sync.dma_start(wk_sb[:, 4:8, :], WKT[:, 4:8, :])
            nc.sync.dma_start(xs0[:, 4:6, :], XT4[0, :, 4:6, :])
            nc.sync.dma_start(wk_sb[:, 8:16, :], WKT[:, 8:16, :])
            nc.sync.dma_start(xs0[:, 6:8, :], XT4[0, :, 6:8, :])
            nc.sync.dma_start(wk_sb[:, 16:32, :], WKT[:, 16:32, :])
            for q in range(4):
                nc.sync.dma_start(wq_sb[:, 0, q * 8:(q + 1) * 8, :],
                                  WQT[:, 0, q * 8:(q + 1) * 8, :])
            nc.sync.dma_start(xs0[:, 8:12, :], XT4[0, :, 8:12, :])
            nc.sync.dma_start(cs0[:, 0:2, :],
                              ROPE[0:2, :, 0:PC].rearrange("j p s -> p j s"))
            for hf in (0, 1):
                nc.sync.dma_start(wq_sb[:, 1, hf * 16:(hf + 1) * 16, :],
                                  WQT[:, 1, hf * 16:(hf + 1) * 16, :])
            nc.sync.dma_start(xs0[:, 12:20, :], XT4[0, :, 12:20, :])
            for hf in (0, 1):
                nc.sync.dma_start(wq_sb[:, 2, hf * 16:(hf + 1) * 16, :],
                                  WQT[:, 2, hf * 16:(hf + 1) * 16, :])
            nc.sync.dma_start(xs0[:, 20:32, :], XT4[0, :, 20:32, :])
            for hf in (0, 1):
                nc.sync.dma_start(wq_sb[:, 3, hf * 16:(hf + 1) * 16, :],
                                  WQT[:, 3, hf * 16:(hf + 1) * 16, :])
            nc.sync.dma_start(wv_sb[:, 0:16, :], WVT[:, 0:16, :])
            nc.sync.dma_start(wv_sb[:, 16:32, :], WVT[:, 16:32, :])

            tri_sb = cpool.tile([128, 4, AC], dt.bfloat16)
            nc.sync.dma_start(tri_sb[:], TRI.rearrange("j p f -> p j f"))
            ones_col = cpool.tile([128, 1], dt.bfloat16)
            nc.vector.memset(ones_col[:], 1.0)


            # --- persistent activations
            qT_sb = qkvpool.tile([128, G, S], dt.bfloat16)     # per-head [d, s]
            kT_sb = qkvpool.tile([128, S], dt.bfloat16)        # [d, s]
            v_sb = qkvpool.tile([128, N_ST, D], dt.bfloat16)   # per s-tile [t, d]

            # =========== attention chunk machinery (phase 2 ops) ===========
            attnTs = {}          # chunk -> {h: attnT tile}

            class AttChunk:
                def __init__(self, c, ep):
                    self.c = c
                    self.ep = ep
                    self.nb = 4 * (c + 1) if causal else N_ST
                    self.np = self.nb // 2
                    self.q0 = c * AC
                    self.e = {}
                    self.esum = {}
                    self.ot = {}
                    self.den = {}
                    self.recip = {}
                    self.bcsb = {}
                    attnTs[c] = {}

                def col0(self, b):
                    # first valid column of key-block b for this q chunk:
                    # diagonal blocks only contribute to q columns >= 128*delta
                    d0 = 4 * self.c
                    return 128 * (b - d0) if (causal and b > d0) else 0

                def opA(self, h, p):
                    def f():
                        d0 = 4 * self.c
                        st = ps_st.tile([128, 2, AC], dt.float32, tag="st", name="st")
                        parts = []
                        for i in (0, 1):
                            b = 2 * p + i
                            c0 = self.col0(b)
                            nc.tensor.matmul(
                                st[:, i, c0:], kT_sb[:, b * 128:(b + 1) * 128],
                                qT_sb[:, h, self.q0 + c0:self.q0 + AC],
                                start=True, stop=True)
                            parts.append((i, b, c0))
                        e = self.ep.tile([128, 2, AC], dt.bfloat16, tag="e", name="e")
                        if all(c0 == 0 for _, _, c0 in parts):
                            nc.scalar.activation(e[:], st[:],
                                                 mybir.ActivationFunctionType.Exp)
                        else:
                            for i, b, c0 in parts:
                                nc.scalar.activation(
                                    e[:, i, c0:], st[:, i, c0:],
                                    mybir.ActivationFunctionType.Exp)
                        for i, b, c0 in parts:
                            if causal and b >= d0:
                                # only the 128-col partial-triangle band needs
                                # masking; cols >= c0+128 are fully valid
                                nc.vector.tensor_mul(
                                    e[:, i, c0:c0 + 128], e[:, i, c0:c0 + 128],
                                    tri_sb[:, b - d0, c0:c0 + 128])
                        with nc.allow_low_precision(reason="softmax denom bf16"):
                            if p == 0:
                                es = esumpool.tile([128, AC], dt.bfloat16,
                                                   tag="esum", name="esum")
                                self.esum[h] = es
                                (_, _, ca), (_, _, cb) = parts
                                if cb == 0:
                                    nc.vector.tensor_add(es[:], e[:, 0, :],
                                                         e[:, 1, :])
                                else:
                                    nc.vector.tensor_copy(es[:], e[:, 0, :])
                                    nc.vector.tensor_add(es[:, cb:], es[:, cb:],
                                                         e[:, 1, cb:])
                            else:
                                es = self.esum[h]
                                for i, b, c0 in parts:
                                    nc.vector.tensor_add(es[:, c0:], es[:, c0:],
                                                         e[:, i, c0:])
                        self.e[(h, p)] = e
                    return f

                def opB(self, h, p):
                    def f():
                        if p == 0:
                            self.ot[h] = ps_ot.tile([128, AC], dt.float32,
                                                    tag="ot", name="ot")
                        ot = self.ot[h]
                        e = self.e.pop((h, p))
                        for i in (0, 1):
                            b = 2 * p + i
                            # diagonal blocks: cols < 128*(b-d0) are fully
                            # masked (e==0) -> skip them in the PV matmul.
                            # b==0 always covers all cols so start init is
                            # complete.
                            c0 = self.col0(b)
                            nc.tensor.matmul(ot[:, c0:AC], v_sb[:, b, :],
                                             e[:, i, c0:AC],
                                             start=(b == 0),
                                             stop=(b == self.nb - 1))
                    return f

                def opDen(self, h):
                    def f():
                        den = ps_den.tile([1, AC], dt.float32, tag="den", name="den")
                        nc.tensor.matmul(den[:], ones_col[:], self.esum[h][:],
                                         start=True, stop=True)
                        recip = recippool.tile([1, AC], dt.float32, tag="recip", name="recip")
                        with nc.allow_low_precision(reason="softmax recip"):
                            nc.vector.reciprocal_approx_fast(recip[:], den[:])
                        self.den[h] = den
                        self.recip[h] = recip
                    return f

                def opBc(self, h):
                    def f():
                        bcsb = bcpool.tile([128, AC], dt.float32, tag="bc", name="bcsb")
                        nc.gpsimd.partition_broadcast(bcsb[:], self.recip[h][:],
                                                      channels=128)
                        self.bcsb[h] = bcsb
                    return f

                def opAttnT(self, h):
                    def f():
                        at = attnpool.tile([128, AC], dt.bfloat16, tag="attnT", name="attnT")
                        nc.vector.tensor_mul(at[:], self.ot[h][:],
                                             self.bcsb[h][:])
                        attnTs[self.c][h] = at
                    return f

                def ops(self):
                    L = []
                    for p in range(self.np):
                        L.append(self.opA(0, p))
                    for u in range(1, G):
                        for p in range(self.np):
                            L.append(self.opA(u, p))
                            L.append(self.opB(u - 1, p))
                            if p == 0:
                                L.append(self.opDen(u - 1))
                            if p == 1:
                                L.append(self.opBc(u - 1))
                        L.append(self.opAttnT(u - 1))
                    for p in range(self.np):
                        L.append(self.opB(G - 1, p))
                        if p == 0:
                            L.append(self.opDen(G - 1))
                        if p == 1:
                            L.append(self.opBc(G - 1))
                    L.append(self.opAttnT(G - 1))
                    return L

            # o_proj per chunk: list of per-tile callables (set up in S4)
            oproj_state = {}

            def oproj_ops(c, wo_sb, outpool, ps_po):
                L = []
                q0 = c * AC

                def mk(t, pc, idx):
                    def f():
                        po = ps_po.tile([128, OSH], dt.float32, tag="po", name="po")
                        ats = attnTs[c]
                        for hh in range(G):
                            nc.tensor.matmul(
                                po[:], ats[hh][:, t * 128:(t + 1) * 128],
                                wo_sb[:, hh, pc * OSH:(pc + 1) * OSH],
                                start=(hh == 0), stop=(hh == G - 1))
                        o_sb = outpool.tile([128, OSH], dt.bfloat16, tag="o", name="o_sb")
                        mod = 2 if c == 3 else 4
                        if idx % mod != mod - 1:
                            nc.vector.tensor_copy(o_sb[:], po[:])
                        else:
                            nc.scalar.copy(o_sb[:], po[:])
                        nc.sync.dma_start(
                            OUT[q0 + t * 128: q0 + (t + 1) * 128,
                                pc * OSH:(pc + 1) * OSH], o_sb[:])
                    return f

                # pc-major so the first po tiles need only the first wo
                # column chunks (wo DMA is split to match)
                for pc in range(8):
                    for t in range(AC // 128):
                        L.append(mk(t, pc, pc * 4 + t))
                return L

            # ============== Phase 1 with attention pump ====================
            pending = []

            def pump(n):
                for _ in range(min(n, len(pending))):
                    pending.pop(0)()

            for sc in range(N_PC):
                if causal and sc >= 1:
                    pending.extend(AttChunk(sc - 1, epool).ops())
                # budget: drain this chunk's pending over ~24 gap points
                gap_budget = max(1, -(-len(pending) // 24))

                s0 = sc * PC
                if sc == 0:
                    xs, cs = xs0, cs0
                else:
                    xs = xspool.tile([128, NE, PC], dt.bfloat16, tag="xs")
                    for q in range(4):
                        nc.sync.dma_start(xs[:, q * 8:(q + 1) * 8, :],
                                          XT4[sc, :, q * 8:(q + 1) * 8, :])
                    cs = cspool.tile([128, 4, PC], dt.bfloat16, tag="cs")
                    nc.sync.dma_start(cs[:], ROPE[:, :, s0:s0 + PC]
                                      .rearrange("j p s -> p j s"))
                cosq_t, sinq_t = cs[:, 0, :], cs[:, 1, :]
                cosk_t, sink_t = cs[:, 2, :], cs[:, 3, :]

                # chunk 0: k, q0..q3, v (wq streams in per head, wv last);
                # later chunks: k, v, q (all weights resident, order free)
                order = ([G] + list(range(G)) + [-1]) if sc == 0 \
                    else ([G, -1] + list(range(G)))
                for hh in order:
                    if hh == -1:
                        for u in range(PC // 128):
                            st = (s0 // 128) + u
                            pv = ps_p1.tile([128, D], dt.float32, tag="p1")
                            for e in range(NE):
                                nc.tensor.matmul(pv[:],
                                                 xs[:, e, u * 128:(u + 1) * 128],
                                                 wv_sb[:, e, :],
                                                 start=(e == 0), stop=(e == NE - 1))
                            nc.vector.tensor_copy(v_sb[:, st, :], pv[:])
                            pump(gap_budget)
                        continue
                    is_k = hh == G
                    pq = ps_p1.tile([128, PC], dt.float32, tag="p1")
                    for e in range(NE):
                        lhsT = (wk_sb[:, e, :] if is_k
                                else wq_sb[:, hh, e, :])
                        nc.tensor.matmul(pq[:], lhsT, xs[:, e, :],
                                         start=(e == 0), stop=(e == NE - 1))
                        if e % 8 == 7 and e != NE - 1:
                            pump(gap_budget)
                    cos_t, sin_t = (cosk_t, sink_t) if is_k else (cosq_t, sinq_t)
                    dest = kT_sb[:, s0:s0 + PC] if is_k \
                        else qT_sb[:, hh, s0:s0 + PC]
                    rt = rtmppool.tile([128, 2, PC], dt.float32, tag="rt")
                    t1, t2 = rt[:, 0, :], rt[:, 1, :]
                    # low half: q'= q_lo*cos_lo + q_hi*sin_mod_lo
                    nc.vector.tensor_mul(t1[0:64, :], pq[0:64, :], cos_t[0:64, :])
                    nc.vector.tensor_mul(t2[0:64, :], pq[64:128, :], sin_t[0:64, :])
                    nc.vector.tensor_add(dest[0:64, :], t1[0:64, :], t2[0:64, :])
                    # high half: q'= q_hi*cos_hi + q_lo*sin_mod_hi
                    nc.vector.tensor_mul(t1[64:128, :], pq[64:128, :], cos_t[64:128, :])
                    nc.vector.tensor_mul(t2[64:128, :], pq[0:64, :], sin_t[64:128, :])
                    nc.vector.tensor_add(dest[64:128, :], t1[64:128, :], t2[64:128, :])
                    pump(gap_budget)

            while pending:
                pending.pop(0)()
            p1ctx.close()

            # ============== S4: attention c3 + o_proj c0-c2 ================
            s4ctx = ExitStack()
            wopool = s4ctx.enter_context(tc.tile_pool(name="wop", bufs=1))
            e3pool = s4ctx.enter_context(tc.tile_pool(name="e3", bufs=18))
            outpool = s4ctx.enter_context(tc.tile_pool(name="outs", bufs=8))
            ps_po = s4ctx.enter_context(
                tc.tile_pool(name="ps_po", bufs=3, space="PSUM"))

            wo_sb = wopool.tile([128, G, E], dt.bfloat16)
            for q in range(16):
                nc.sync.dma_start(wo_sb[:, :, q * 256:(q + 1) * 256],
                                  WOT[:, :, q * 256:(q + 1) * 256])

            if causal:
                ops3 = AttChunk(3, e3pool).ops()
                poq = []
                for c in range(3):
                    poq.extend(oproj_ops(c, wo_sb, outpool, ps_po))
                for i, op in enumerate(ops3):
                    op()
                    if i >= 14:
                        for _ in range(2):
                            if poq:
                                poq.pop(0)()
                while poq:
                    poq.pop(0)()
                for op in oproj_ops(3, wo_sb, outpool, ps_po):
                    op()
            else:
                # dense: attention needs full kT/v, so chunks run here,
                # zipped with the previous chunk's o_proj
                poq = []
                for c in range(N_AC):
                    ops_c = AttChunk(c, e3pool).ops()
                    for i, op in enumerate(ops_c):
                        op()
                        for _ in range(2):
                            if poq:
                                poq.pop(0)()
                    poq.extend(oproj_ops(c, wo_sb, outpool, ps_po))
                while poq:
                    poq.pop(0)()

            s4ctx.close()

    nc.compile()
    _BUILD_CACHE[key] = nc
    return nc


def _prep_inputs(hidden_states, attention_mask, cos, sin, Wq, Wk, Wv, Wo):
    X = np.asarray(hidden_states, dtype=np.float32).reshape(S, E)
    # [N_PC, 128, NE, PC]: exact SBUF tile layout per chunk -> long DMA runs
    XT4 = np.ascontiguousarray(
        X.reshape(N_PC, PC, NE, 128).transpose(0, 3, 2, 1)).astype(BF16)

    m = np.asarray(attention_mask, dtype=np.float32).reshape(S, S)
    il, ju = np.tril_indices(S), np.triu_indices(S, 1)
    causal = bool(np.all(m[il] == 0.0) and np.all(m[ju] <= -1e8))
    dense = bool(np.all(m == 0.0))
    if not (causal or dense):
        raise NotImplementedError("only causal or all-zero masks supported")

    scale = 1.0 / math.sqrt(D)
    cosT = np.ascontiguousarray(np.asarray(cos, np.float32).reshape(S, D).T)
    sinT = np.ascontiguousarray(np.asarray(sin, np.float32).reshape(S, D).T)
    sin_mod = sinT.copy()
    sin_mod[0:64] *= -1.0
    rope_t = np.stack([cosT * scale, sin_mod * scale, cosT, sin_mod]) \
        .astype(BF16)

    p = np.arange(128)[:, None]
    f = np.arange(AC)[None, :]
    tri = np.stack([(128 * jj + p <= f) for jj in range(4)]).astype(BF16)

    Wq = np.asarray(Wq, np.float32)
    Wk = np.asarray(Wk, np.float32)
    Wv = np.asarray(Wv, np.float32)
    Wo = np.asarray(Wo, np.float32)

    def wtile(Wshard):
        # [out, E] -> SBUF layout [128, NE, out]
        return np.ascontiguousarray(
            Wshard.T.reshape(NE, 128, Wshard.shape[0]).transpose(1, 0, 2)
        ).astype(BF16)

    in_maps = []
    for c in range(N_CORES):
        in_maps.append({
            "xt": XT4,
            "wqt": np.ascontiguousarray(np.stack(
                [wtile(Wq[c * OSH + hh * D:c * OSH + (hh + 1) * D, :])
                 for hh in range(G)], axis=1)),
            "wkt": wtile(Wk[c * D:(c + 1) * D, :]),
            "wvt": wtile(Wv[c * D:(c + 1) * D, :]),
            "wot": np.ascontiguousarray(
                Wo[:, c * OSH:(c + 1) * OSH].T.reshape(G, 128, E)
                .transpose(1, 0, 2)).astype(BF16),
            "rope": rope_t,
            "tri": tri,
        })
    return in_maps, causal


def kernel(hidden_states, attention_mask, cos, sin, Wq, Wk, Wv, Wo,
           _trace=False, _tmpdir=None):
    in_maps, causal = _prep_inputs(hidden_states, attention_mask, cos, sin,
                                   Wq, Wk, Wv, Wo)
    nc = build_bass(causal)
    res = run_bass_kernel_spmd(nc, in_maps, core_ids=list(range(N_CORES)),
                               trace=_trace, tmpdir=_tmpdir)
    out = res.results[0]["out"].astype(np.float32)
    for c in range(1, N_CORES):
        out = out + res.results[c]["out"].astype(np.float32)
    kernel._last_result = res
    return out.reshape(1, S, E).astype(np.float32)


# revision 38
# speedup vs baseline: 1.0114x; 1.0114x over previous
"""Trainium2 Bass kernel for Mllama-style GQA self-attention (B=1, S=2048,
H=32 q-heads, KVH=8 kv-heads, D=128), tensor-parallel over heads across 8
NeuronCores.

Sharding: core c owns kv-head c and q-heads 4c..4c+3 (Wq/Wk/Wv column shards),
computes its heads' attention output in transposed [feature, seq] layout, then
computes a partial output projection over its 512 features producing full
4096-wide rows in bf16; the host sums the 8 cores' partials in fp32.

Global software pipeline: attention for seq-chunk c (scores, exp, PV) is
emitted interleaved into phase-1 chunk c+1's projection matmul stream, so the
exp/mask latency chains hide under independent PE work and the PE stays
continuously busy (keeping its p-state at max). Scores are issued in 2-block
pairs into a 2-bank PSUM tile so one Act-engine exp instruction covers both
blocks. o_proj for chunks 0-2 interleaves with chunk 3's attention; softmax
normalization uses reciprocal_approx_fast and a GpSimd partition broadcast;
diagonal (causal-boundary) blocks compute only their valid column range.
"""
import math
from contextlib import ExitStack
import numpy as np
import ml_dtypes

import concourse.bass as bass
import concourse.bacc as bacc
import concourse.mybir as mybir
import concourse.tile as tile
from concourse.bass_utils import run_bass_kernel_spmd

BF16 = ml_dtypes.bfloat16
S, E, H, KVH, D = 2048, 4096, 32, 8, 128
N_CORES = 8
G = H // KVH                      # q heads per core
OSH = G * D                       # per-core q/attn feature count (512)
PC = 512                          # phase-1 seq chunk (projection rhs width)
AC = 512                          # attention sq chunk width
N_PC = S // PC                    # 4
N_AC = S // AC                    # 4
NE = E // 128                     # 32 contraction tiles
N_ST = S // 128                   # 16 seq tiles

_BUILD_CACHE = {}


def build_bass(causal: bool):
    key = causal
    if key in _BUILD_CACHE:
        return _BUILD_CACHE[key]
    dt = mybir.dt
    nc = bacc.Bacc("TRN2", target_bir_lowering=False, debug=False,
                   enable_asserts=False, num_devices=N_CORES)

    XT4 = nc.dram_tensor("xt", [N_PC, 128, NE, PC], dt.bfloat16, kind="ExternalInput").ap()
    WQT = nc.dram_tensor("wqt", [128, NE, OSH], dt.bfloat16, kind="ExternalInput").ap()
    WKT = nc.dram_tensor("wkt", [128, NE, D], dt.bfloat16, kind="ExternalInput").ap()
    WVT = nc.dram_tensor("wvt", [128, NE, D], dt.bfloat16, kind="ExternalInput").ap()
    WOT = nc.dram_tensor("wot", [128, G, E], dt.bfloat16, kind="ExternalInput").ap()
    ROPE = nc.dram_tensor("rope", [4, D, S], dt.bfloat16, kind="ExternalInput").ap()
    TRI = nc.dram_tensor("tri", [4, 128, AC], dt.bfloat16, kind="ExternalInput").ap()
    OUT = nc.dram_tensor("out", [S, E], dt.bfloat16, kind="ExternalOutput").ap()

    with tile.TileContext(nc) as tc:
        with (
            tc.tile_pool(name="wpool", bufs=1) as wpool,
            tc.tile_pool(name="qkv", bufs=1) as qkvpool,
            tc.tile_pool(name="consts", bufs=1) as cpool,
            tc.tile_pool(name="epool", bufs=13) as epool,
            tc.tile_pool(name="esump", bufs=3) as esumpool,
            tc.tile_pool(name="recipp", bufs=2) as recippool,
            tc.tile_pool(name="bcp", bufs=2) as bcpool,
            tc.tile_pool(name="attn", bufs=16) as attnpool,
            tc.tile_pool(name="ps_st", bufs=1, space="PSUM") as ps_st,
            tc.tile_pool(name="ps_ot", bufs=2, space="PSUM") as ps_ot,
            tc.tile_pool(name="ps_den", bufs=1, space="PSUM") as ps_den,
        ):
            # phase-1-only pools; closed after phase 1 so wo/e3/out pools
            # can reuse their SBUF space (and ps_po the PSUM bank pair)
            p1ctx = ExitStack()
            xspool = p1ctx.enter_context(tc.tile_pool(name="xs", bufs=2))
            cspool = p1ctx.enter_context(tc.tile_pool(name="cs", bufs=2))
            rtmppool = p1ctx.enter_context(tc.tile_pool(name="rtmp", bufs=1))
            ps_p1 = p1ctx.enter_context(
                tc.tile_pool(name="ps_p1", bufs=2, space="PSUM"))

            # --- resident weights. Priority: first chunk's activations and
            # Wk jump ahead of the big weight burst so the PE starts early.
            wq_sb = wpool.tile([128, NE, OSH], dt.bfloat16)
            wk_sb = wpool.tile([128, NE, D], dt.bfloat16)
            wv_sb = wpool.tile([128, NE, D], dt.bfloat16)

            xs0 = xspool.tile([128, NE, PC], dt.bfloat16, tag="xs")
            cs0 = cspool.tile([128, 4, PC], dt.bfloat16, tag="cs")
            # first e-tiles split single so the k chain's first steps aren't
            # gated on one DMA queue moving a 512KB group
            for q in range(4):
                nc.sync.dma_start(xs0[:, q:q + 1, :], XT4[0, :, q:q + 1, :])
                nc.sync.dma_start(wk_sb[:, q:q + 1, :], WKT[:, q:q + 1, :])
            nc.sync.dma_start(xs0[:, 4:6, :], XT4[0, :, 4:6, :])
            nc.sync.dma_start(wk_sb[:, 4:8, :], WKT[:, 4:8, :])
            nc.sync.dma_start(xs0[:, 6:8, :], XT4[0, :, 6:8, :])
            nc.sync.dma_start(wk_sb[:, 8:16, :], WKT[:, 8:16, :])
            nc.sync.dma_start(xs0[:, 8:12, :], XT4[0, :, 8:12, :])
            nc.sync.dma_start(wk_sb[:, 16:32, :], WKT[:, 16:32, :])
            for q in range(3, 8):
                nc.sync.dma_start(xs0[:, q * 4:(q + 1) * 4, :],
                                  XT4[0, :, q * 4:(q + 1) * 4, :])
            for q in range(8):
                nc.sync.dma_start(wv_sb[:, q * 4:(q + 1) * 4, :],
                                  WVT[:, q * 4:(q + 1) * 4, :])
            nc.sync.dma_start(cs0[:], ROPE[:, :, 0:PC].rearrange("j p s -> p j s"))
            for q in range(8):
                nc.sync.dma_start(wq_sb[:, q * 4:(q + 1) * 4, :],
                                  WQT[:, q * 4:(q + 1) * 4, :])

            tri_sb = cpool.tile([128, 4, AC], dt.bfloat16)
            nc.sync.dma_start(tri_sb[:], TRI.rearrange("j p f -> p j f"))
            ones_col = cpool.tile([128, 1], dt.bfloat16)
            nc.vector.memset(ones_col[:], 1.0)


            # --- persistent activations
            qT_sb = qkvpool.tile([128, G, S], dt.bfloat16)     # per-head [d, s]
            kT_sb = qkvpool.tile([128, S], dt.bfloat16)        # [d, s]
            v_sb = qkvpool.tile([128, N_ST, D], dt.bfloat16)   # per s-tile [t, d]

            # =========== attention chunk machinery (phase 2 ops) ===========
            attnTs = {}          # chunk -> {h: attnT tile}

            class AttChunk:
                def __init__(self, c, ep):
                    self.c = c
                    self.ep = ep
                    self.nb = 4 * (c + 1) if causal else N_ST
                    self.np = self.nb // 2
                    self.q0 = c * AC
                    self.e = {}
                    self.esum = {}
                    self.ot = {}
                    self.den = {}
                    self.recip = {}
                    self.bcsb = {}
                    attnTs[c] = {}

                def col0(self, b):
                    # first valid column of key-block b for this q chunk:
                    # diagonal blocks only contribute to q columns >= 128*delta
                    d0 = 4 * self.c
                    return 128 * (b - d0) if (causal and b > d0) else 0

                def opA(self, h, p):
                    def f():
                        d0 = 4 * self.c
                        st = ps_st.tile([128, 2, AC], dt.float32, tag="st", name="st")
                        parts = []
                        for i in (0, 1):
                            b = 2 * p + i
                            c0 = self.col0(b)
                            nc.tensor.matmul(
                                st[:, i, c0:], kT_sb[:, b * 128:(b + 1) * 128],
                                qT_sb[:, h, self.q0 + c0:self.q0 + AC],
                                start=True, stop=True)
                            parts.append((i, b, c0))
                        e = self.ep.tile([128, 2, AC], dt.bfloat16, tag="e", name="e")
                        if all(c0 == 0 for _, _, c0 in parts):
                            nc.scalar.activation(e[:], st[:],
                                                 mybir.ActivationFunctionType.Exp)
                        else:
                            for i, b, c0 in parts:
                                nc.scalar.activation(
                                    e[:, i, c0:], st[:, i, c0:],
                                    mybir.ActivationFunctionType.Exp)
                        for i, b, c0 in parts:
                            if causal and b >= d0:
                                # only the 128-col partial-triangle band needs
                                # masking; cols >= c0+128 are fully valid
                                nc.vector.tensor_mul(
                                    e[:, i, c0:c0 + 128], e[:, i, c0:c0 + 128],
                                    tri_sb[:, b - d0, c0:c0 + 128])
                        with nc.allow_low_precision(reason="softmax denom bf16"):
                            if p == 0:
                                es = esumpool.tile([128, AC], dt.bfloat16,
                                                   tag="esum", name="esum")
                                self.esum[h] = es
                                (_, _, ca), (_, _, cb) = parts
                                if cb == 0:
                                    nc.vector.tensor_add(es[:], e[:, 0, :],
                                                         e[:, 1, :])
                                else:
                                    nc.vector.tensor_copy(es[:], e[:, 0, :])
                                    nc.vector.tensor_add(es[:, cb:], es[:, cb:],
                                                         e[:, 1, cb:])
                            else:
                                es = self.esum[h]
                                for i, b, c0 in parts:
                                    nc.vector.tensor_add(es[:, c0:], es[:, c0:],
                                                         e[:, i, c0:])
                        self.e[(h, p)] = e
                    return f

                def opB(self, h, p):
                    def f():
                        if p == 0:
                            self.ot[h] = ps_ot.tile([128, AC], dt.float32,
                                                    tag="ot", name="ot")
                        ot = self.ot[h]
                        e = self.e.pop((h, p))
                        for i in (0, 1):
                            b = 2 * p + i
                            # diagonal blocks: cols < 128*(b-d0) are fully
                            # masked (e==0) -> skip them in the PV matmul.
                            # b==0 always covers all cols so start init is
                            # complete.
                            c0 = self.col0(b)
                            nc.tensor.matmul(ot[:, c0:AC], v_sb[:, b, :],
                                             e[:, i, c0:AC],
                                             start=(b == 0),
                                             stop=(b == self.nb - 1))
                    return f

                def opDen(self, h):
                    def f():
                        den = ps_den.tile([1, AC], dt.float32, tag="den", name="den")
                        nc.tensor.matmul(den[:], ones_col[:], self.esum[h][:],
                                         start=True, stop=True)
                        recip = recippool.tile([1, AC], dt.float32, tag="recip", name="recip")
                        with nc.allow_low_precision(reason="softmax recip"):
                            nc.vector.reciprocal_approx_fast(recip[:], den[:])
                        self.den[h] = den
                        self.recip[h] = recip
                    return f

                def opBc(self, h):
                    def f():
                        bcsb = bcpool.tile([128, AC], dt.float32, tag="bc", name="bcsb")
                        nc.gpsimd.partition_broadcast(bcsb[:], self.recip[h][:],
                                                      channels=128)
                        self.bcsb[h] = bcsb
                    return f

                def opAttnT(self, h):
                    def f():
                        at = attnpool.tile([128, AC], dt.bfloat16, tag="attnT", name="attnT")
                        nc.vector.tensor_mul(at[:], self.ot[h][:],
                                             self.bcsb[h][:])
                        attnTs[self.c][h] = at
                    return f

                def ops(self):
                    L = []
                    for p in range(self.np):
                        L.append(self.opA(0, p))
                    for u in range(1, G):
                        for p in range(self.np):
                            L.append(self.opA(u, p))
                            L.append(self.opB(u - 1, p))
                            if p == 0:
                                L.append(self.opDen(u - 1))
                            if p == 1:
                                L.append(self.opBc(u - 1))
                        L.append(self.opAttnT(u - 1))
                    for p in range(self.np):
                        L.append(self.opB(G - 1, p))
                        if p == 0:
                            L.append(self.opDen(G - 1))
                        if p == 1:
                            L.append(self.opBc(G - 1))
                    L.append(self.opAttnT(G - 1))
                    return L

            # o_proj per chunk: list of per-tile callables (set up in S4)
            oproj_state = {}

            def oproj_ops(c, wo_sb, outpool, ps_po):
                L = []
                q0 = c * AC

                def mk(t, pc, idx):
                    def f():
                        po = ps_po.tile([128, OSH], dt.float32, tag="po", name="po")
                        ats = attnTs[c]
                        for hh in range(G):
                            nc.tensor.matmul(
                                po[:], ats[hh][:, t * 128:(t + 1) * 128],
                                wo_sb[:, hh, pc * OSH:(pc + 1) * OSH],
                                start=(hh == 0), stop=(hh == G - 1))
                        o_sb = outpool.tile([128, OSH], dt.bfloat16, tag="o", name="o_sb")
                        mod = 2 if c == 3 else 4
                        if idx % mod != mod - 1:
                            nc.vector.tensor_copy(o_sb[:], po[:])
                        else:
                            nc.scalar.copy(o_sb[:], po[:])
                        nc.sync.dma_start(
                            OUT[q0 + t * 128: q0 + (t + 1) * 128,
                                pc * OSH:(pc + 1) * OSH], o_sb[:])
                    return f

                # pc-major so the first po tiles need only the first wo
                # column chunks (wo DMA is split to match)
                for pc in range(8):
                    for t in range(AC // 128):
                        L.append(mk(t, pc, pc * 4 + t))
                return L

            # ============== Phase 1 with attention pump ====================
            pending = []

            def pump(n):
                for _ in range(min(n, len(pending))):
                    pending.pop(0)()

            for sc in range(N_PC):
                if causal and sc >= 1:
                    pending.extend(AttChunk(sc - 1, epool).ops())
                # budget: drain this chunk's pending over ~24 gap points
                gap_budget = max(1, -(-len(pending) // 24))

                s0 = sc * PC
                if sc == 0:
                    xs, cs = xs0, cs0
                else:
                    xs = xspool.tile([128, NE, PC], dt.bfloat16, tag="xs")
                    for q in range(4):
                        nc.sync.dma_start(xs[:, q * 8:(q + 1) * 8, :],
                                          XT4[sc, :, q * 8:(q + 1) * 8, :])
                    cs = cspool.tile([128, 4, PC], dt.bfloat16, tag="cs")
                    nc.sync.dma_start(cs[:], ROPE[:, :, s0:s0 + PC]
                                      .rearrange("j p s -> p j s"))
                cosq_t, sinq_t = cs[:, 0, :], cs[:, 1, :]
                cosk_t, sink_t = cs[:, 2, :], cs[:, 3, :]

                # k head first, then v, then q heads (k/v weights land first)
                for hh in [G, -1] + list(range(G)):
                    if hh == -1:
                        for u in range(PC // 128):
                            st = (s0 // 128) + u
                            pv = ps_p1.tile([128, D], dt.float32, tag="p1")
                            for e in range(NE):
                                nc.tensor.matmul(pv[:],
                                                 xs[:, e, u * 128:(u + 1) * 128],
                                                 wv_sb[:, e, :],
                                                 start=(e == 0), stop=(e == NE - 1))
                            nc.vector.tensor_copy(v_sb[:, st, :], pv[:])
                            pump(gap_budget)
                        continue
                    is_k = hh == G
                    pq = ps_p1.tile([128, PC], dt.float32, tag="p1")
                    for e in range(NE):
                        lhsT = (wk_sb[:, e, :] if is_k
                                else wq_sb[:, e, hh * D:(hh + 1) * D])
                        nc.tensor.matmul(pq[:], lhsT, xs[:, e, :],
                                         start=(e == 0), stop=(e == NE - 1))
                        if e % 8 == 7 and e != NE - 1:
                            pump(gap_budget)
                    cos_t, sin_t = (cosk_t, sink_t) if is_k else (cosq_t, sinq_t)
                    dest = kT_sb[:, s0:s0 + PC] if is_k \
                        else qT_sb[:, hh, s0:s0 + PC]
                    rt = rtmppool.tile([128, 2, PC], dt.float32, tag="rt")
                    t1, t2 = rt[:, 0, :], rt[:, 1, :]
                    # low half: q'= q_lo*cos_lo + q_hi*sin_mod_lo
                    nc.vector.tensor_mul(t1[0:64, :], pq[0:64, :], cos_t[0:64, :])
                    nc.vector.tensor_mul(t2[0:64, :], pq[64:128, :], sin_t[0:64, :])
                    nc.vector.tensor_add(dest[0:64, :], t1[0:64, :], t2[0:64, :])
                    # high half: q'= q_hi*cos_hi + q_lo*sin_mod_hi
                    nc.vector.tensor_mul(t1[64:128, :], pq[64:128, :], cos_t[64:128, :])
                    nc.vector.tensor_mul(t2[64:128, :], pq[0:64, :], sin_t[64:128, :])
                    nc.vector.tensor_add(dest[64:128, :], t1[64:128, :], t2[64:128, :])
                    pump(gap_budget)

            while pending:
                pending.pop(0)()
            p1ctx.close()

            # ============== S4: attention c3 + o_proj c0-c2 ================
            s4ctx = ExitStack()
            wopool = s4ctx.enter_context(tc.tile_pool(name="wop", bufs=1))
            e3pool = s4ctx.enter_context(tc.tile_pool(name="e3", bufs=18))
            outpool = s4ctx.enter_context(tc.tile_pool(name="outs", bufs=8))
            ps_po = s4ctx.enter_context(
                tc.tile_pool(name="ps_po", bufs=3, space="PSUM"))

            wo_sb = wopool.tile([128, G, E], dt.bfloat16)
            for q in range(16):
                nc.sync.dma_start(wo_sb[:, :, q * 256:(q + 1) * 256],
                                  WOT[:, :, q * 256:(q + 1) * 256])

            if causal:
                ops3 = AttChunk(3, e3pool).ops()
                poq = []
                for c in range(3):
                    poq.extend(oproj_ops(c, wo_sb, outpool, ps_po))
                for i, op in enumerate(ops3):
                    op()
                    if i >= 14:
                        for _ in range(2):
                            if poq:
                                poq.pop(0)()
                while poq:
                    poq.pop(0)()
                for op in oproj_ops(3, wo_sb, outpool, ps_po):
                    op()
            else:
                # dense: attention needs full kT/v, so chunks run here,
                # zipped with the previous chunk's o_proj
                poq = []
                for c in range(N_AC):
                    ops_c = AttChunk(c, e3pool).ops()
                    for i, op in enumerate(ops_c):
                        op()
                        for _ in range(2):
                            if poq:
                                poq.pop(0)()
                    poq.extend(oproj_ops(c, wo_sb, outpool, ps_po))
                while poq:
                    poq.pop(0)()

            s4ctx.close()

    nc.compile()
    _BUILD_CACHE[key] = nc
    return nc


def _prep_inputs(hidden_states, attention_mask, cos, sin, Wq, Wk, Wv, Wo):
    X = np.asarray(hidden_states, dtype=np.float32).reshape(S, E)
    # [N_PC, 128, NE, PC]: exact SBUF tile layout per chunk -> long DMA runs
    XT4 = np.ascontiguousarray(
        X.reshape(N_PC, PC, NE, 128).transpose(0, 3, 2, 1)).astype(BF16)

    m = np.asarray(attention_mask, dtype=np.float32).reshape(S, S)
    il, ju = np.tril_indices(S), np.triu_indices(S, 1)
    causal = bool(np.all(m[il] == 0.0) and np.all(m[ju] <= -1e8))
    dense = bool(np.all(m == 0.0))
    if not (causal or dense):
        raise NotImplementedError("only causal or all-zero masks supported")

    scale = 1.0 / math.sqrt(D)
    cosT = np.ascontiguousarray(np.asarray(cos, np.float32).reshape(S, D).T)
    sinT = np.ascontiguousarray(np.asarray(sin, np.float32).reshape(S, D).T)
    sin_mod = sinT.copy()
    sin_mod[0:64] *= -1.0
    rope_t = np.stack([cosT * scale, sin_mod * scale, cosT, sin_mod]) \
        .astype(BF16)

    p = np.arange(128)[:, None]
    f = np.arange(AC)[None, :]
    tri = np.stack([(128 * jj + p <= f) for jj in range(4)]).astype(BF16)

    Wq = np.asarray(Wq, np.float32)
    Wk = np.asarray(Wk, np.float32)
    Wv = np.asarray(Wv, np.float32)
    Wo = np.asarray(Wo, np.float32)

    def wtile(Wshard):
        # [out, E] -> SBUF layout [128, NE, out]
        return np.ascontiguousarray(
            Wshard.T.reshape(NE, 128, Wshard.shape[0]).transpose(1, 0, 2)
        ).astype(BF16)

    in_maps = []
    for c in range(N_CORES):
        in_maps.append({
            "xt": XT4,
            "wqt": wtile(Wq[c * OSH:(c + 1) * OSH, :]),
            "wkt": wtile(Wk[c * D:(c + 1) * D, :]),
            "wvt": wtile(Wv[c * D:(c + 1) * D, :]),
            "wot": np.ascontiguousarray(
                Wo[:, c * OSH:(c + 1) * OSH].T.reshape(G, 128, E)
                .transpose(1, 0, 2)).astype(BF16),
            "rope": rope_t,
            "tri": tri,
        })
    return in_maps, causal


def kernel(hidden_states, attention_mask, cos, sin, Wq, Wk, Wv, Wo,
           _trace=False, _tmpdir=None):
    in_maps, causal = _prep_inputs(hidden_states, attention_mask, cos, sin,
                                   Wq, Wk, Wv, Wo)
    nc = build_bass(causal)
    res = run_bass_kernel_spmd(nc, in_maps, core_ids=list(range(N_CORES)),
                               trace=_trace, tmpdir=_tmpdir)
    out = res.results[0]["out"].astype(np.float32)
    for c in range(1, N_CORES):
        out = out + res.results[c]["out"].astype(np.float32)
    kernel._last_result = res
    return out.reshape(1, S, E).astype(np.float32)
